# revision 1
# baseline (speedup 1.0000x reference)
"""GCN 2-layer encoder kernel for Trainium2, 8 NeuronCores.

out = PReLU(LN(D^-1/2 (A+I) D^-1/2 X W + b)) twice.

Host side: fold source-side degree norm into the gather table
(x' = dinv[n]*x[n], bf16); partition edges by destination owner
(6250 nodes/core); sort by dst into 128-dst windows; within each
window split edges by source half (lo: src<32768, hi: src>=32768) so
int16-indexed dma_gather can address the table through two overlapping
views; pack into 128-edge chunks, pad with dst_rel=-1 (one-hot never
matches -> zero contribution).

Device side (SPMD x8): per gather block (<=8 chunks = 1024 edges) one
dma_gather streams source rows into SBUF at ~3.5ns/row; per chunk an
is_equal against a constant iota row builds the one-hot scatter matrix
and a TensorE matmul scatter-adds into the window's PSUM accumulator:
aggT[cols, dst] += msg[edges, cols]^T @ onehot. Per window: W applied
after aggregation ((A@X)@W), dinv[dst] folded into the PSUM->SBUF copy
on the ACT engine, then batched LayerNorm+PReLU over all windows on
DVE. Between layers an AllGather rebuilds the full (bf16) gather table.
"""
import sys
import numpy as np

sys.path.insert(0, "/opt/trn_rl_repo")

import ml_dtypes  # noqa: E402
import concourse.bass as bass  # noqa: E402
import concourse.bacc as bacc  # noqa: E402
import concourse.tile as tile  # noqa: E402
from concourse import mybir  # noqa: E402
from concourse import bass_utils  # noqa: E402
from concourse import library_config  # noqa: E402

N_NODES = 50000
CIN = 128
HID = 96
N_CORES = 8
NPC = N_NODES // N_CORES  # 6250 nodes per core
P = 128
NW = (NPC + P - 1) // P  # 49 windows per core
TAIL_ROWS = NPC - (NW - 1) * P  # 106 rows in the last window
EPS = 1e-5
LO_LIM = 32768  # int16 index limit; lo view rows [0, 32768)
HI_BASE = 24576  # hi view = table[24576:], idx = src - HI_BASE
MAXCH = 8  # chunks per dma_gather block (<=1024 indices)
BCOL = MAXCH * P // 16  # idx16 columns reserved per block

F32 = mybir.dt.float32
BF16 = mybir.dt.bfloat16
I16 = mybir.dt.int16
F16 = mybir.dt.float16

_ALU = mybir.AluOpType
_ACTF = mybir.ActivationFunctionType

bf = ml_dtypes.bfloat16


GROUP = 4  # windows per group; chunks pack densely per (group, view)


def plan_blocks(gcnts, seg_wins):
    """Block plan from per-(group,view) chunk counts.

    gcnts: list of ((g0, view), n_chunks) in stream order.
    seg_wins: per global chunk id, sorted list of windows it may span
      (union over cores). Returns (blocks, chunk_g0, win_segs) where
      blocks[b] = (view, [chunk ids]), chunk_g0[k] = group base window,
      win_segs[w] = ordered list of (chunk id, w) segment positions as
      global segment indices (seg id = position in emission order).
    """
    blocks = []
    chunk_g0 = []
    k = 0
    for (g0, view), n_ch in gcnts:
        ks = list(range(k, k + n_ch))
        chunk_g0.extend([g0] * n_ch)
        k += n_ch
        for i in range(0, n_ch, MAXCH):
            blocks.append((view, ks[i : i + MAXCH]))
    # emission order of segments: by chunk, then window
    seg_first_last = {}
    s = 0
    for kk in range(len(chunk_g0)):
        for w in seg_wins[kk]:
            if w not in seg_first_last:
                seg_first_last[w] = [s, s]
            else:
                seg_first_last[w][1] = s
            s += 1
    return blocks, chunk_g0, seg_first_last


def _preprocess(x, edge_index):
    src = edge_index[0].astype(np.int64)
    dst = edge_index[1].astype(np.int64)
    loop = np.arange(N_NODES, dtype=np.int64)
    src = np.concatenate([src, loop])
    dst = np.concatenate([dst, loop])

    deg = np.bincount(dst, minlength=N_NODES).astype(np.float64)
    dinv = (1.0 / np.sqrt(deg)).astype(np.float32)

    x_table = np.zeros((N_NODES + 1, CIN), bf)
    x_table[:N_NODES] = (x * dinv[:, None]).astype(bf)

    core = dst // NPC
    per_core = []
    for p in range(N_CORES):
        m = core == p
        s_p = src[m]
        d_p = dst[m] - p * NPC
        order = np.argsort(d_p, kind="stable")
        per_core.append((s_p[order], d_p[order]))

    # per-window lo/hi edge lists per core; chunk counts maxed over cores
    cnts_lo = np.zeros(NW, np.int64)
    cnts_hi = np.zeros(NW, np.int64)
    win_edges = []  # [core][w] = (lo_src, hi_src, lo_dst_rel, hi_dst_rel)
    for p in range(N_CORES):
        s_p, d_p = per_core[p]
        w_all = d_p // P
        bc = np.bincount(w_all, minlength=NW)
        starts = np.zeros(NW + 1, np.int64)
        starts[1:] = np.cumsum(bc)
        rows = []
        for w in range(NW):
            e0, e1 = int(starts[w]), int(starts[w + 1])
            s_w = s_p[e0:e1]
            d_w = (d_p[e0:e1] - w * P).astype(np.float32)
            m_lo = s_w < LO_LIM
            lo_s, hi_s = s_w[m_lo], s_w[~m_lo]
            lo_d, hi_d = d_w[m_lo], d_w[~m_lo]
            rows.append((lo_s, hi_s, lo_d, hi_d))
            cnts_lo[w] = max(cnts_lo[w], (len(lo_s) + P - 1) // P)
            cnts_hi[w] = max(cnts_hi[w], (len(hi_s) + P - 1) // P)
        win_edges.append(rows)

    # dense per-(group, view) packing; chunks may straddle windows
    n_pw_lo = np.zeros((N_CORES, NW), np.int64)
    n_pw_hi = np.zeros((N_CORES, NW), np.int64)
    for p in range(N_CORES):
        for w in range(NW):
            lo_s, hi_s, _, _ = win_edges[p][w]
            n_pw_lo[p, w] = len(lo_s)
            n_pw_hi[p, w] = len(hi_s)

    gcnts = []
    for g0 in range(0, NW, GROUP):
        ws = list(range(g0, min(g0 + GROUP, NW)))
        for view, n_pw in (("lo", n_pw_lo), ("hi", n_pw_hi)):
            tot = n_pw[:, ws].sum(axis=1)  # per core
            n_ch = int((tot.max() + P - 1) // P)
            if n_ch > 0:
                gcnts.append(((g0, view), n_ch))

    C = sum(n for _, n in gcnts)
    # per-core fills + union of windows spanned per chunk
    seg_win_sets = [set() for _ in range(C)]
    chunk_idx_all = np.zeros((N_CORES, C, P), np.int64)
    dst_rel = np.full((N_CORES, P, C), -1.0, np.float16)
    for p in range(N_CORES):
        k0 = 0
        for (g0, view), n_ch in gcnts:
            ws = list(range(g0, min(g0 + GROUP, NW)))
            if view == "lo":
                segs = [(w, win_edges[p][w][0], win_edges[p][w][2], 0)
                        for w in ws]
            else:
                segs = [(w, win_edges[p][w][1], win_edges[p][w][3], HI_BASE)
                        for w in ws]
            idx_cat = np.concatenate(
                [s - base for (_, s, _, base) in segs] or [np.zeros(0, np.int64)])
            d_cat = np.concatenate(
                [d + (w - g0) * P for (w, _, d, _) in segs]
                or [np.zeros(0, np.float32)])
            w_cat = np.concatenate(
                [np.full(len(s), w) for (w, s, _, _) in segs]
                or [np.zeros(0, np.int64)])
            n = len(idx_cat)
            cap = n_ch * P
            assert n <= cap, (n, cap)
            pad = cap - n
            idx_pad = np.concatenate([idx_cat, np.zeros(pad, np.int64)])
            chunk_idx_all[p, k0 : k0 + n_ch] = idx_pad.reshape(n_ch, P)
            i = np.arange(n)
            dst_rel[p, i % P, k0 + i // P] = d_cat.astype(np.float16)
            for kk in range(n_ch):
                wset = set(w_cat[kk * P : (kk + 1) * P].tolist())
                seg_win_sets[k0 + kk] |= wset
            k0 += n_ch

    # every window must appear in at least one segment (PSUM init + finish)
    seen = set().union(*seg_win_sets) if seg_win_sets else set()
    for w in range(NW):
        if w not in seen:
            g = w // GROUP
            for kk, (gv, n_ch) in enumerate(gcnts):
                pass
            # attach to the first chunk of the window's group's lo stream
            k0 = 0
            for (g0, view), n_ch in gcnts:
                if g0 == (w // GROUP) * GROUP and view == "lo":
                    seg_win_sets[k0].add(w)
                    break
                k0 += n_ch
    seg_wins = [sorted(s) for s in seg_win_sets]

    blocks, chunk_g0, seg_first_last = plan_blocks(gcnts, seg_wins)
    NB = len(blocks)
    idx16 = np.zeros((N_CORES, P, NB * BCOL), np.int16)
    for p in range(N_CORES):
        for b, (view, ks) in enumerate(blocks):
            nidx = len(ks) * P
            flat = chunk_idx_all[p, ks].reshape(nidx)
            wrap = flat.reshape(nidx // 16, 16).T.astype(np.int16)
            idx16[p, :, b * BCOL : b * BCOL + nidx // 16] = np.tile(wrap, (8, 1))

    dinv_own = np.zeros((N_CORES, P, NW), np.float32)
    for p in range(N_CORES):
        v = dinv[p * NPC : (p + 1) * NPC]
        pad = np.zeros(NW * P, np.float32)
        pad[:NPC] = v
        dinv_own[p] = pad.reshape(NW, P).T

    iota = np.tile(np.arange(512, dtype=np.float32), (P, 1)).astype(np.float16)
    plan = (gcnts, seg_wins)
    return x_table, idx16, dst_rel, dinv_own, iota, plan


def _bcast_mid(ap2d, mid):
    """[128, F] AP -> [128, mid, F] with stride-0 middle dim."""
    return bass.AP(
        tensor=ap2d.tensor,
        offset=ap2d.offset,
        ap=[ap2d.ap[0], [0, mid], ap2d.ap[1]],
    )


def build_program(plan, loop_R=None, include_ag=True, ablate=(),
                  single_packet=False, msg_bufs=24):
    gcnts, seg_wins = plan
    blocks, chunk_g0, seg_first_last = plan_blocks(gcnts, seg_wins)
    C = len(chunk_g0)
    NB = len(blocks)

    nc = bacc.Bacc("TRN2", target_bir_lowering=False, debug=False,
                   enable_asserts=True, num_devices=N_CORES)

    x_table = nc.dram_tensor("x_table", [N_NODES + 1, CIN], BF16,
                             kind="ExternalInput")
    idx16_d = nc.dram_tensor("idx16", [P, NB * BCOL], I16, kind="ExternalInput")
    dst_rel_d = nc.dram_tensor("dst_rel", [P, C], F16, kind="ExternalInput")
    dinv_own_d = nc.dram_tensor("dinv_own", [P, NW], F32, kind="ExternalInput")
    iota_d = nc.dram_tensor("iota", [P, 4 * P], F16, kind="ExternalInput")
    w1_d = nc.dram_tensor("w1", [CIN, HID], F32, kind="ExternalInput")
    w2p_d = nc.dram_tensor("w2p", [CIN, HID], F32, kind="ExternalInput")
    # rows: b1, g1, be1, b2, g2, be2, a
    vecs_d = nc.dram_tensor("vecs", [7, HID], F32, kind="ExternalInput")
    out_d = nc.dram_tensor("out", [NPC, HID], F32, kind="ExternalOutput")

    with tile.TileContext(nc) as tc:
        with tc.tile_pool(name="const", bufs=1) as const, \
             tc.tile_pool(name="msgp", bufs=msg_bufs) as msgp, \
             tc.tile_pool(name="selp", bufs=8) as selp, \
             tc.tile_pool(name="aggp", bufs=2) as aggp, \
             tc.tile_pool(name="bigp", bufs=1) as bigp, \
             tc.tile_pool(name="stat", bufs=4) as statp, \
             tc.tile_pool(name="psum", bufs=2, space="PSUM") as psum, \
             tc.tile_pool(name="dram", bufs=1, space="DRAM") as dram:

            nc.gpsimd.load_library(library_config.mlp)

            idx16_t = const.tile([P, NB * BCOL], I16)
            nc.sync.dma_start(out=idx16_t[:], in_=idx16_d[:])
            dst_rel = const.tile([P, C], F16)
            nc.sync.dma_start(out=dst_rel[:], in_=dst_rel_d[:])
            dinv_own = const.tile([P, NW], F32)
            nc.sync.dma_start(out=dinv_own[:], in_=dinv_own_d[:])
            iota_t = const.tile([P, 4 * P], F16)
            nc.sync.dma_start(out=iota_t[:], in_=iota_d[:])
            w1_t = const.tile([CIN, HID], F32)
            nc.sync.dma_start(out=w1_t[:], in_=w1_d[:])
            w2p_t = const.tile([CIN, HID], F32)
            nc.sync.dma_start(out=w2p_t[:], in_=w2p_d[:])

            vt = []
            for i in range(7):
                v = const.tile([P, HID], F32, name=f"vec{i}")
                nc.sync.dma_start(
                    out=v[:],
                    in_=bass.AP(tensor=vecs_d, offset=i * HID,
                                ap=[[0, P], [1, HID]]),
                )
                vt.append(v)
            b1_t, g1_t, be1_t, b2_t, g2_t, be2_t, a_t = vt

            eps_t = const.tile([P, 1], F32)
            nc.vector.memset(eps_t[:], EPS)

            table2 = dram.tile([N_NODES + 1, CIN], BF16)
            ag_in = dram.tile([NPC, CIN], BF16)

            zrow = const.tile([1, CIN], BF16)
            nc.vector.memset(zrow[:], 0.0)
            nc.sync.dma_start(out=table2[N_NODES : N_NODES + 1, :], in_=zrow[:])

            const_msg = const_sel = None
            if "gather" in ablate:
                const_msg = const.tile([P, MAXCH, CIN], BF16)
                nc.gpsimd.memset(const_msg[:], 0.125)
            if "sel" in ablate:
                const_sel = const.tile([P, P], BF16)
                nc.vector.memset(const_sel[:], 0.007)

            def aggregate_layer(table_t, w_t, layer):
                """One layer: gather+scatter-add agg, W, bias, LN, PReLU.
                Returns batch tile with output in [:, :, 0:HID]."""
                batch = bigp.tile([P, NW, HID], F32, tag="batch",
                                  name=f"batch{layer}")
                mv = bigp.tile([P, NW, 2], F32, tag="mv", name=f"mv{layer}")
                sd = bigp.tile([P, NW, 1], F32, tag="sd", name=f"sd{layer}")
                rstd = bigp.tile([P, NW, 1], F32, tag="rstd", name=f"rstd{layer}")
                v_t = bigp.tile([P, NW, HID], F32, tag="vt", name=f"v{layer}")

                view_lo = table_t[0:LO_LIM, :]
                view_hi = table_t[HI_BASE : N_NODES + 1, :]

                b_t = b1_t if layer == 1 else b2_t
                g_t = g1_t if layer == 1 else g2_t
                be_t = be1_t if layer == 1 else be2_t

                def finish_window(w, aggT):
                    aggT_sb = aggp.tile([P, P], F32, tag="aggsb",
                                        name=f"aggsb{layer}_{w}")
                    nc.scalar.activation(out=aggT_sb[:], in_=aggT[:],
                                         func=_ACTF.Copy)
                    h_ps = psum.tile([P, HID], F32, tag="hps", space="PSUM",
                                     name=f"hps{layer}_{w}")
                    nc.tensor.matmul(out=h_ps[:], lhsT=aggT_sb[:], rhs=w_t[:],
                                     start=True, stop=True)
                    nc.scalar.activation(
                        out=batch[:, w, :],
                        in_=h_ps[:],
                        func=_ACTF.Copy,
                        scale=dinv_own[:, w : w + 1],
                    )

                aggT_of = {}
                seg_id = 0
                for b, (view, ks) in enumerate(blocks):
                    n_ch = len(ks)
                    nidx = n_ch * P
                    if const_msg is not None:
                        msg3 = const_msg
                    else:
                        msg3 = msgp.tile([P, MAXCH, CIN], BF16, tag="msg",
                                         name=f"msg{layer}_{b}")
                        src_view = view_lo if view == "lo" else view_hi
                        nc.gpsimd.dma_gather(
                            msg3[:, :n_ch, :],
                            src_view,
                            idx16_t[:, b * BCOL : b * BCOL + nidx // 16],
                            nidx, nidx, CIN,
                            single_packet=single_packet,
                        )
                    for s, k in enumerate(ks):
                        g0 = chunk_g0[k]
                        for w in seg_wins[k]:
                            s_first, s_last = seg_first_last[w]
                            if seg_id == s_first:
                                aggT_of[w] = psum.tile(
                                    [P, P], F32, tag="aggT", space="PSUM",
                                    bufs=6, name=f"aggT{layer}_{w}")
                            aggT = aggT_of[w]
                            if const_sel is not None:
                                sel = const_sel
                            else:
                                sel = selp.tile([P, P], BF16, tag="sel",
                                                name=f"sel{layer}_{seg_id}")
                                off = (w - g0) * P
                                nc.vector.tensor_tensor(
                                    out=sel[:],
                                    in0=dst_rel[:, k : k + 1].to_broadcast([P, P]),
                                    in1=iota_t[:, off : off + P],
                                    op=_ALU.is_equal,
                                )
                            if "mm" not in ablate:
                                nc.tensor.matmul(
                                    out=aggT[:],
                                    lhsT=msg3[:, s, :],
                                    rhs=sel[:],
                                    start=(seg_id == s_first),
                                    stop=(seg_id == s_last),
                                )
                            elif seg_id == s_first or seg_id == s_last:
                                nc.tensor.matmul(
                                    out=aggT[:], lhsT=msg3[:, min(s, n_ch - 1), :],
                                    rhs=sel[:],
                                    start=(seg_id == s_first),
                                    stop=(seg_id == s_last),
                                )
                            if seg_id == s_last:
                                finish_window(w, aggT_of.pop(w))
                            seg_id += 1

                # ---- batched bias + LN + PReLU ----
                hview = batch[:, :, :]
                nc.vector.tensor_tensor(out=hview, in0=hview,
                                        in1=_bcast_mid(b_t[:], NW), op=_ALU.add)
                for w in range(NW):
                    st = statp.tile([P, 6], F32, tag="bnst",
                                    name=f"bnst{layer}_{w}")
                    nc.vector.bn_stats(out=st[:], in_=batch[:, w, :])
                    nc.vector.bn_aggr(out=mv[:, w, :], in_=st[:])
                nc.scalar.activation(out=sd[:, :, 0], in_=mv[:, :, 1],
                                     func=_ACTF.Sqrt, bias=eps_t[:], scale=1.0)
                nc.vector.reciprocal(out=rstd[:, :, 0], in_=sd[:, :, 0])
                nc.vector.tensor_tensor(
                    out=hview, in0=hview,
                    in1=mv[:, :, 0:1].to_broadcast([P, NW, HID]),
                    op=_ALU.subtract)
                nc.vector.tensor_tensor(
                    out=hview, in0=hview,
                    in1=rstd[:].to_broadcast([P, NW, HID]), op=_ALU.mult)
                nc.vector.tensor_tensor(out=hview, in0=hview,
                                        in1=_bcast_mid(g_t[:], NW), op=_ALU.mult)
                nc.vector.tensor_tensor(out=hview, in0=hview,
                                        in1=_bcast_mid(be_t[:], NW), op=_ALU.add)
                nc.vector.scalar_tensor_tensor(out=v_t[:], in0=hview, scalar=0.0,
                                               in1=_bcast_mid(a_t[:], NW),
                                               op0=_ALU.min, op1=_ALU.mult)
                nc.vector.scalar_tensor_tensor(out=hview, in0=hview, scalar=0.0,
                                               in1=v_t[:],
                                               op0=_ALU.max, op1=_ALU.add)
                return batch

            def body1():
                batch1 = aggregate_layer(x_table, w1_t, layer=1)
                # bf16 staging for the layer-2 table, with source-side dinv
                fin = bigp.tile([P, NW, CIN], BF16, tag="fin", name="fin1")
                nc.gpsimd.memset(fin[:, :, HID:CIN], 0.0)
                dinv3 = bass.AP(
                    tensor=dinv_own.tensor, offset=dinv_own.offset,
                    ap=[dinv_own.ap[0], dinv_own.ap[1], [0, HID]])
                nc.vector.tensor_tensor(out=fin[:, :, 0:HID],
                                        in0=batch1[:, :, :], in1=dinv3,
                                        op=_ALU.mult)
                nc.sync.dma_start(
                    out=ag_in[0 : (NW - 1) * P, :].rearrange(
                        "(w j) f -> j w f", j=P),
                    in_=fin[:, 0 : NW - 1, :],
                )
                nc.sync.dma_start(
                    out=ag_in[(NW - 1) * P : NPC, :],
                    in_=fin[0:TAIL_ROWS, NW - 1, :],
                )

            def body2():
                batch2 = aggregate_layer(table2, w2p_t, layer=2)
                nc.sync.dma_start(
                    out=out_d[0 : (NW - 1) * P, :].rearrange(
                        "(w j) f -> j w f", j=P),
                    in_=batch2[:, 0 : NW - 1, :],
                )
                nc.sync.dma_start(
                    out=out_d[(NW - 1) * P : NPC, :],
                    in_=batch2[0:TAIL_ROWS, NW - 1, :],
                )

            def ag():
                nc.gpsimd.collective_compute(
                    "AllGather",
                    _ALU.bypass,
                    ins=[ag_in[:, :]],
                    outs=[table2[0:N_NODES, :]],
                    replica_groups=[list(range(N_CORES))],
                )

            if loop_R is None:
                body1()
                if include_ag:
                    ag()
                body2()
            elif loop_R < 0:
                # python-unrolled chain of -loop_R full iterations (AG incl.)
                for _ in range(-loop_R):
                    body1()
                    if include_ag:
                        ag()
                    body2()
            else:
                if include_ag:
                    body1()
                    ag()
                with tc.For_i(0, loop_R, 1):
                    body1()
                    body2()

    nc.compile()
    return nc


def _make_in_maps(x, W1, b1, W2, b2, g1, be1, g2, be2, a, pre):
    x_table, idx16, dst_rel, dinv_own, iota, plan = pre
    w2p = np.zeros((CIN, HID), np.float32)
    w2p[:HID] = W2
    vecs = np.stack([b1, g1, be1, b2, g2, be2, a]).astype(np.float32)
    in_maps = []
    for p in range(N_CORES):
        in_maps.append({
            "x_table": x_table,
            "idx16": idx16[p],
            "dst_rel": dst_rel[p],
            "dinv_own": dinv_own[p],
            "iota": iota,
            "w1": np.asarray(W1, np.float32),
            "w2p": w2p,
            "vecs": vecs,
        })
    return in_maps, plan


def kernel(x, edge_index, W1, b1, W2, b2, g1, be1, g2, be2, a):
    x = np.asarray(x, np.float32)
    edge_index = np.asarray(edge_index)
    pre = _preprocess(x, edge_index)
    in_maps, plan = _make_in_maps(
        x, np.asarray(W1), np.asarray(b1), np.asarray(W2), np.asarray(b2),
        np.asarray(g1), np.asarray(be1), np.asarray(g2), np.asarray(be2),
        np.asarray(a), pre)
    nc = build_program(plan)
    res = bass_utils.run_bass_kernel_spmd(nc, in_maps, core_ids=list(range(N_CORES)))
    out = np.concatenate([res.results[p]["out"] for p in range(N_CORES)], axis=0)
    return out.astype(np.float32)



# revision 9
# speedup vs baseline: 2.6933x; 2.6933x over previous
"""GCN 2-layer encoder kernel for Trainium2, 8 NeuronCores.

out = PReLU(LN(D^-1/2 (A+I) D^-1/2 X W + b)) twice.

Host side: fold source-side degree norm into the gather table
(x' = dinv[n]*x[n], bf16); partition edges by destination owner
(6250 nodes/core); sort by dst into 128-dst windows; within each
window split edges by source half (lo: src<32768, hi: src>=32768) so
int16-indexed dma_gather can address the table through two overlapping
views; pack into 128-edge chunks, pad with dst_rel=-1 (one-hot never
matches -> zero contribution).

Device side (SPMD x8): per gather block (<=8 chunks = 1024 edges) one
dma_gather streams source rows into SBUF at ~3.5ns/row; per chunk an
is_equal against a constant iota row builds the one-hot scatter matrix
and a TensorE matmul scatter-adds into the window's PSUM accumulator:
aggT[cols, dst] += msg[edges, cols]^T @ onehot. Per window: W applied
after aggregation ((A@X)@W), dinv[dst] folded into the PSUM->SBUF copy
on the ACT engine, then batched LayerNorm+PReLU over all windows on
DVE. Between layers an AllGather rebuilds the full (bf16) gather table.
"""
import sys
import numpy as np

sys.path.insert(0, "/opt/trn_rl_repo")

import ml_dtypes  # noqa: E402
import concourse.bass as bass  # noqa: E402
import concourse.bacc as bacc  # noqa: E402
import concourse.tile as tile  # noqa: E402
from concourse import mybir  # noqa: E402
from concourse import bass_utils  # noqa: E402
from concourse import library_config  # noqa: E402

N_NODES = 50000
CIN = 128
HID = 96
N_CORES = 8
NPC = N_NODES // N_CORES  # 6250 nodes per core
P = 128
NW = (NPC + P - 1) // P  # 49 windows per core
TAIL_ROWS = NPC - (NW - 1) * P  # 106 rows in the last window
EPS = 1e-5
LO_LIM = 32768  # int16 index limit; lo view rows [0, 32768)
HI_BASE = 24576  # hi view = table[24576:], idx = src - HI_BASE
MAXCH = 8  # chunks per dma_gather block (<=1024 indices)
BCOL = MAXCH * P // 16  # idx16 columns reserved per block

F32 = mybir.dt.float32
BF16 = mybir.dt.bfloat16
I16 = mybir.dt.int16
F16 = mybir.dt.float16

_ALU = mybir.AluOpType
_ACTF = mybir.ActivationFunctionType

bf = ml_dtypes.bfloat16


GROUP = 4  # windows per group; chunks pack densely per (group, view)


def plan_blocks(gcnts, seg_wins):
    """Block plan from per-(group,view) chunk counts.

    gcnts: list of ((g0, view), n_chunks) in stream order.
    seg_wins: per global chunk id, sorted list of windows it may span
      (union over cores). Returns (blocks, chunk_g0, win_segs) where
      blocks[b] = (view, [chunk ids]), chunk_g0[k] = group base window,
      win_segs[w] = ordered list of (chunk id, w) segment positions as
      global segment indices (seg id = position in emission order).
    """
    blocks = []
    chunk_g0 = []
    k = 0
    for (g0, view), n_ch in gcnts:
        ks = list(range(k, k + n_ch))
        chunk_g0.extend([g0] * n_ch)
        k += n_ch
        for i in range(0, n_ch, MAXCH):
            blocks.append((view, ks[i : i + MAXCH]))
    # emission order of segments: by chunk, then window
    seg_first_last = {}
    s = 0
    for kk in range(len(chunk_g0)):
        for w in seg_wins[kk]:
            if w not in seg_first_last:
                seg_first_last[w] = [s, s]
            else:
                seg_first_last[w][1] = s
            s += 1
    return blocks, chunk_g0, seg_first_last


def _preprocess(x, edge_index):
    src = edge_index[0].astype(np.int64)
    dst = edge_index[1].astype(np.int64)
    loop = np.arange(N_NODES, dtype=np.int64)
    src = np.concatenate([src, loop])
    dst = np.concatenate([dst, loop])

    deg = np.bincount(dst, minlength=N_NODES).astype(np.float64)
    dinv = (1.0 / np.sqrt(deg)).astype(np.float32)

    x_table = np.zeros((N_NODES + 1, CIN), bf)
    x_table[:N_NODES] = (x * dinv[:, None]).astype(bf)

    core = dst // NPC
    per_core = []
    for p in range(N_CORES):
        m = core == p
        s_p = src[m]
        d_p = dst[m] - p * NPC
        order = np.argsort(d_p, kind="stable")
        per_core.append((s_p[order], d_p[order]))

    # per-window lo/hi edge lists per core; chunk counts maxed over cores
    cnts_lo = np.zeros(NW, np.int64)
    cnts_hi = np.zeros(NW, np.int64)
    win_edges = []  # [core][w] = (lo_src, hi_src, lo_dst_rel, hi_dst_rel)
    for p in range(N_CORES):
        s_p, d_p = per_core[p]
        w_all = d_p // P
        bc = np.bincount(w_all, minlength=NW)
        starts = np.zeros(NW + 1, np.int64)
        starts[1:] = np.cumsum(bc)
        rows = []
        for w in range(NW):
            e0, e1 = int(starts[w]), int(starts[w + 1])
            s_w = s_p[e0:e1]
            d_w = (d_p[e0:e1] - w * P).astype(np.float32)
            m_lo = s_w < LO_LIM
            lo_s, hi_s = s_w[m_lo], s_w[~m_lo]
            lo_d, hi_d = d_w[m_lo], d_w[~m_lo]
            rows.append((lo_s, hi_s, lo_d, hi_d))
            cnts_lo[w] = max(cnts_lo[w], (len(lo_s) + P - 1) // P)
            cnts_hi[w] = max(cnts_hi[w], (len(hi_s) + P - 1) // P)
        win_edges.append(rows)

    # dense per-(group, view) packing; chunks may straddle windows
    n_pw_lo = np.zeros((N_CORES, NW), np.int64)
    n_pw_hi = np.zeros((N_CORES, NW), np.int64)
    for p in range(N_CORES):
        for w in range(NW):
            lo_s, hi_s, _, _ = win_edges[p][w]
            n_pw_lo[p, w] = len(lo_s)
            n_pw_hi[p, w] = len(hi_s)

    gcnts = []
    for g0 in range(0, NW, GROUP):
        ws = list(range(g0, min(g0 + GROUP, NW)))
        for view, n_pw in (("lo", n_pw_lo), ("hi", n_pw_hi)):
            tot = n_pw[:, ws].sum(axis=1)  # per core
            n_ch = int((tot.max() + P - 1) // P)
            if n_ch > 0:
                gcnts.append(((g0, view), n_ch))

    C = sum(n for _, n in gcnts)
    # per-core fills + union of windows spanned per chunk
    seg_win_sets = [set() for _ in range(C)]
    chunk_idx_all = np.zeros((N_CORES, C, P), np.int64)
    dst_rel = np.full((N_CORES, P, C), -1.0, np.float16)
    for p in range(N_CORES):
        k0 = 0
        for (g0, view), n_ch in gcnts:
            ws = list(range(g0, min(g0 + GROUP, NW)))
            if view == "lo":
                segs = [(w, win_edges[p][w][0], win_edges[p][w][2], 0)
                        for w in ws]
            else:
                segs = [(w, win_edges[p][w][1], win_edges[p][w][3], HI_BASE)
                        for w in ws]
            idx_cat = np.concatenate(
                [s - base for (_, s, _, base) in segs] or [np.zeros(0, np.int64)])
            d_cat = np.concatenate(
                [d + (w - g0) * P for (w, _, d, _) in segs]
                or [np.zeros(0, np.float32)])
            w_cat = np.concatenate(
                [np.full(len(s), w) for (w, s, _, _) in segs]
                or [np.zeros(0, np.int64)])
            n = len(idx_cat)
            cap = n_ch * P
            assert n <= cap, (n, cap)
            pad = cap - n
            idx_pad = np.concatenate([idx_cat, np.zeros(pad, np.int64)])
            chunk_idx_all[p, k0 : k0 + n_ch] = idx_pad.reshape(n_ch, P)
            i = np.arange(n)
            dst_rel[p, i % P, k0 + i // P] = d_cat.astype(np.float16)
            for kk in range(n_ch):
                wset = set(w_cat[kk * P : (kk + 1) * P].tolist())
                seg_win_sets[k0 + kk] |= wset
            k0 += n_ch

    # every window must appear in at least one segment (PSUM init + finish)
    seen = set().union(*seg_win_sets) if seg_win_sets else set()
    for w in range(NW):
        if w not in seen:
            g = w // GROUP
            for kk, (gv, n_ch) in enumerate(gcnts):
                pass
            # attach to the first chunk of the window's group's lo stream
            k0 = 0
            for (g0, view), n_ch in gcnts:
                if g0 == (w // GROUP) * GROUP and view == "lo":
                    seg_win_sets[k0].add(w)
                    break
                k0 += n_ch
    seg_wins = [sorted(s) for s in seg_win_sets]

    blocks, chunk_g0, seg_first_last = plan_blocks(gcnts, seg_wins)
    NB = len(blocks)
    idx16 = np.zeros((N_CORES, P, NB * BCOL), np.int16)
    for p in range(N_CORES):
        for b, (view, ks) in enumerate(blocks):
            nidx = len(ks) * P
            flat = chunk_idx_all[p, ks].reshape(nidx)
            wrap = flat.reshape(nidx // 16, 16).T.astype(np.int16)
            idx16[p, :, b * BCOL : b * BCOL + nidx // 16] = np.tile(wrap, (8, 1))

    dinv_own = np.zeros((N_CORES, P, NW), np.float32)
    for p in range(N_CORES):
        v = dinv[p * NPC : (p + 1) * NPC]
        pad = np.zeros(NW * P, np.float32)
        pad[:NPC] = v
        dinv_own[p] = pad.reshape(NW, P).T

    iota = np.tile(np.arange(512, dtype=np.float32), (P, 1)).astype(np.float16)
    plan = (gcnts, seg_wins)
    return x_table, idx16, dst_rel, dinv_own, iota, plan


def _bcast_mid(ap2d, mid):
    """[128, F] AP -> [128, mid, F] with stride-0 middle dim."""
    return bass.AP(
        tensor=ap2d.tensor,
        offset=ap2d.offset,
        ap=[ap2d.ap[0], [0, mid], ap2d.ap[1]],
    )


def _bcast_last(ap2d, last):
    """[128, F] AP -> [128, F, last] with stride-0 trailing dim."""
    return bass.AP(
        tensor=ap2d.tensor,
        offset=ap2d.offset,
        ap=[ap2d.ap[0], ap2d.ap[1], [0, last]],
    )


def build_program(plan, loop_R=None, include_ag=True, ablate=(),
                  single_packet=True, msg_bufs=24, nq=4, sel_bufs=6):
    gcnts, seg_wins = plan
    blocks, chunk_g0, seg_first_last = plan_blocks(gcnts, seg_wins)
    C = len(chunk_g0)
    NB = len(blocks)

    # per-stream batched-sel metadata: stream = one (group, view) entry of
    # gcnts; each block belongs to exactly one stream. For each window of a
    # stream, the chunks that may touch it form a contiguous range [kf, kl].
    stream_wr = []  # per stream: dict w -> (kf, kl)
    block_stream = []
    k0 = 0
    bi = 0
    NKMAX = 1
    for (g0, view), n_ch in gcnts:
        wr = {}
        for k in range(k0, k0 + n_ch):
            for w in seg_wins[k]:
                a, b = wr.get(w, (k, k))
                wr[w] = (min(a, k), max(b, k))
        for w, (kf, kl) in wr.items():
            NKMAX = max(NKMAX, kl - kf + 1)
        stream_wr.append(wr)
        for _ in range(0, n_ch, MAXCH):
            block_stream.append(len(stream_wr) - 1)
            bi += 1
        k0 += n_ch

    nc = bacc.Bacc("TRN2", target_bir_lowering=False, debug=False,
                   enable_asserts=True, num_devices=N_CORES,
                   num_swdge_queues=nq)

    x_table = nc.dram_tensor("x_table", [N_NODES + 1, CIN], BF16,
                             kind="ExternalInput")
    idx16_d = nc.dram_tensor("idx16", [P, NB * BCOL], I16, kind="ExternalInput")
    dst_rel_d = nc.dram_tensor("dst_rel", [P, C], F16, kind="ExternalInput")
    dinv_own_d = nc.dram_tensor("dinv_own", [P, NW], F32, kind="ExternalInput")
    iota_d = nc.dram_tensor("iota", [P, 4 * P], F16, kind="ExternalInput")
    w1_d = nc.dram_tensor("w1", [CIN, HID], F32, kind="ExternalInput")
    w2p_d = nc.dram_tensor("w2p", [CIN, HID], F32, kind="ExternalInput")
    # rows: b1, g1, be1, b2, g2, be2, a
    vecs_d = nc.dram_tensor("vecs", [7, HID], F32, kind="ExternalInput")
    out_d = nc.dram_tensor("out", [NPC, HID], F32, kind="ExternalOutput")

    with tile.TileContext(nc) as tc:
        with tc.tile_pool(name="const", bufs=1) as const, \
             tc.tile_pool(name="msgp", bufs=msg_bufs) as msgp, \
             tc.tile_pool(name="selp", bufs=sel_bufs) as selp, \
             tc.tile_pool(name="aggp", bufs=2) as aggp, \
             tc.tile_pool(name="bigp", bufs=1) as bigp, \
             tc.tile_pool(name="stat", bufs=4) as statp, \
             tc.tile_pool(name="psum", bufs=2, space="PSUM") as psum, \
             tc.tile_pool(name="dram", bufs=1, space="DRAM") as dram:

            nc.gpsimd.load_library(library_config.mlp)

            idx16_t = const.tile([P, NB * BCOL], I16)
            nc.sync.dma_start(out=idx16_t[:], in_=idx16_d[:])
            dst_rel = const.tile([P, C], F16)
            nc.sync.dma_start(out=dst_rel[:], in_=dst_rel_d[:])
            dinv_own = const.tile([P, NW], F32)
            nc.sync.dma_start(out=dinv_own[:], in_=dinv_own_d[:])
            iota_t = const.tile([P, 4 * P], F16)
            nc.sync.dma_start(out=iota_t[:], in_=iota_d[:])
            w1_t = const.tile([CIN, HID], F32)
            nc.sync.dma_start(out=w1_t[:], in_=w1_d[:])
            w2p_t = const.tile([CIN, HID], F32)
            nc.sync.dma_start(out=w2p_t[:], in_=w2p_d[:])

            vt = []
            for i in range(7):
                v = const.tile([P, HID], F32, name=f"vec{i}")
                nc.sync.dma_start(
                    out=v[:],
                    in_=bass.AP(tensor=vecs_d, offset=i * HID,
                                ap=[[0, P], [1, HID]]),
                )
                vt.append(v)
            b1_t, g1_t, be1_t, b2_t, g2_t, be2_t, a_t = vt

            eps_t = const.tile([P, 1], F32)
            nc.vector.memset(eps_t[:], EPS)

            table2 = dram.tile([N_NODES + 1, CIN], BF16)
            ag_in = dram.tile([NPC, CIN], BF16)

            zrow = const.tile([1, CIN], BF16)
            nc.vector.memset(zrow[:], 0.0)
            nc.sync.dma_start(out=table2[N_NODES : N_NODES + 1, :], in_=zrow[:])

            const_msg = const_sel = None
            if "gather" in ablate:
                const_msg = const.tile([P, MAXCH, CIN], BF16)
                nc.gpsimd.memset(const_msg[:], 0.125)
            if "sel" in ablate:
                const_sel = const.tile([P, P], BF16)
                nc.vector.memset(const_sel[:], 0.007)

            def aggregate_layer(table_t, w_t, layer):
                """One layer: gather+scatter-add agg, W, bias, LN, PReLU.
                Returns batch tile with output in [:, :, 0:HID]."""
                batch = bigp.tile([P, NW, HID], F32, tag="batch",
                                  name=f"batch{layer}")
                mv = bigp.tile([P, NW, 2], F32, tag="mv", name=f"mv{layer}")
                sd = bigp.tile([P, NW, 1], F32, tag="sd", name=f"sd{layer}")
                rstd = bigp.tile([P, NW, 1], F32, tag="rstd", name=f"rstd{layer}")
                v_t = bigp.tile([P, NW, HID], F32, tag="vt", name=f"v{layer}")

                view_lo = table_t[0:LO_LIM, :]
                view_hi = table_t[HI_BASE : N_NODES + 1, :]

                b_t = b1_t if layer == 1 else b2_t
                g_t = g1_t if layer == 1 else g2_t
                be_t = be1_t if layer == 1 else be2_t

                def finish_window(w, aggT):
                    aggT_sb = aggp.tile([P, P], F32, tag="aggsb",
                                        name=f"aggsb{layer}_{w}")
                    nc.scalar.activation(out=aggT_sb[:], in_=aggT[:],
                                         func=_ACTF.Copy)
                    h_ps = psum.tile([P, HID], F32, tag="hps", space="PSUM",
                                     name=f"hps{layer}_{w}")
                    nc.tensor.matmul(out=h_ps[:], lhsT=aggT_sb[:], rhs=w_t[:],
                                     start=True, stop=True)
                    nc.scalar.activation(
                        out=batch[:, w, :],
                        in_=h_ps[:],
                        func=_ACTF.Copy,
                        scale=dinv_own[:, w : w + 1],
                    )

                aggT_of = {}
                sel_info = {}
                cur_stream = -1
                seg_id = 0
                for b, (view, ks) in enumerate(blocks):
                    si = block_stream[b]
                    if si != cur_stream and const_sel is None:
                        # batched one-hot builds: one is_equal per window of
                        # this (group, view) stream covering its chunk range
                        cur_stream = si
                        (g0s, _vw), _n = gcnts[si]
                        for w in sorted(stream_wr[si]):
                            kf, kl = stream_wr[si][w]
                            n_k = kl - kf + 1
                            t = selp.tile([P, NKMAX, P], BF16, tag="sel",
                                          name=f"selw{layer}_{si}_{w}")
                            off = (w - g0s) * P
                            nc.vector.tensor_tensor(
                                out=t[:, 0:n_k, :],
                                in0=_bcast_last(dst_rel[:, kf : kl + 1], P),
                                in1=_bcast_mid(iota_t[:, off : off + P], n_k),
                                op=_ALU.is_equal,
                            )
                            sel_info[w] = (t, kf)
                    n_ch = len(ks)
                    nidx = n_ch * P
                    if const_msg is not None:
                        msg3 = const_msg
                    else:
                        msg3 = msgp.tile([P, MAXCH, CIN], BF16, tag="msg",
                                         name=f"msg{layer}_{b}")
                        src_view = view_lo if view == "lo" else view_hi
                        nc.gpsimd.dma_gather(
                            msg3[:, :n_ch, :],
                            src_view,
                            idx16_t[:, b * BCOL : b * BCOL + nidx // 16],
                            nidx, nidx, CIN,
                            single_packet=single_packet,
                            queue_num=b % nq,
                        )
                    for s, k in enumerate(ks):
                        g0 = chunk_g0[k]
                        for w in seg_wins[k]:
                            s_first, s_last = seg_first_last[w]
                            if seg_id == s_first:
                                aggT_of[w] = psum.tile(
                                    [P, P], F32, tag="aggT", space="PSUM",
                                    bufs=6, name=f"aggT{layer}_{w}")
                            aggT = aggT_of[w]
                            if const_sel is not None:
                                sel_ap = const_sel[:]
                            else:
                                t, kf = sel_info[w]
                                sel_ap = t[:, k - kf, :]
                            if "mm" not in ablate:
                                nc.tensor.matmul(
                                    out=aggT[:],
                                    lhsT=msg3[:, s, :],
                                    rhs=sel_ap,
                                    start=(seg_id == s_first),
                                    stop=(seg_id == s_last),
                                )
                            elif seg_id == s_first or seg_id == s_last:
                                nc.tensor.matmul(
                                    out=aggT[:], lhsT=msg3[:, min(s, n_ch - 1), :],
                                    rhs=sel_ap,
                                    start=(seg_id == s_first),
                                    stop=(seg_id == s_last),
                                )
                            if seg_id == s_last:
                                finish_window(w, aggT_of.pop(w))
                            seg_id += 1

                # ---- batched bias + LN + PReLU ----
                hview = batch[:, :, :]
                nc.vector.tensor_tensor(out=hview, in0=hview,
                                        in1=_bcast_mid(b_t[:], NW), op=_ALU.add)
                for w in range(NW):
                    st = statp.tile([P, 6], F32, tag="bnst",
                                    name=f"bnst{layer}_{w}")
                    nc.vector.bn_stats(out=st[:], in_=batch[:, w, :])
                    nc.vector.bn_aggr(out=mv[:, w, :], in_=st[:])
                nc.scalar.activation(out=sd[:, :, 0], in_=mv[:, :, 1],
                                     func=_ACTF.Sqrt, bias=eps_t[:], scale=1.0)
                nc.vector.reciprocal(out=rstd[:, :, 0], in_=sd[:, :, 0])
                nc.vector.tensor_tensor(
                    out=hview, in0=hview,
                    in1=mv[:, :, 0:1].to_broadcast([P, NW, HID]),
                    op=_ALU.subtract)
                nc.vector.tensor_tensor(
                    out=hview, in0=hview,
                    in1=rstd[:].to_broadcast([P, NW, HID]), op=_ALU.mult)
                nc.vector.tensor_tensor(out=hview, in0=hview,
                                        in1=_bcast_mid(g_t[:], NW), op=_ALU.mult)
                nc.vector.tensor_tensor(out=hview, in0=hview,
                                        in1=_bcast_mid(be_t[:], NW), op=_ALU.add)
                nc.vector.scalar_tensor_tensor(out=v_t[:], in0=hview, scalar=0.0,
                                               in1=_bcast_mid(a_t[:], NW),
                                               op0=_ALU.min, op1=_ALU.mult)
                nc.vector.scalar_tensor_tensor(out=hview, in0=hview, scalar=0.0,
                                               in1=v_t[:],
                                               op0=_ALU.max, op1=_ALU.add)
                return batch

            def body1():
                batch1 = aggregate_layer(x_table, w1_t, layer=1)
                # bf16 staging for the layer-2 table, with source-side dinv
                fin = bigp.tile([P, NW, CIN], BF16, tag="fin", name="fin1")
                nc.gpsimd.memset(fin[:, :, HID:CIN], 0.0)
                dinv3 = bass.AP(
                    tensor=dinv_own.tensor, offset=dinv_own.offset,
                    ap=[dinv_own.ap[0], dinv_own.ap[1], [0, HID]])
                nc.vector.tensor_tensor(out=fin[:, :, 0:HID],
                                        in0=batch1[:, :, :], in1=dinv3,
                                        op=_ALU.mult)
                nc.sync.dma_start(
                    out=ag_in[0 : (NW - 1) * P, :].rearrange(
                        "(w j) f -> j w f", j=P),
                    in_=fin[:, 0 : NW - 1, :],
                )
                nc.sync.dma_start(
                    out=ag_in[(NW - 1) * P : NPC, :],
                    in_=fin[0:TAIL_ROWS, NW - 1, :],
                )

            def body2():
                batch2 = aggregate_layer(table2, w2p_t, layer=2)
                nc.sync.dma_start(
                    out=out_d[0 : (NW - 1) * P, :].rearrange(
                        "(w j) f -> j w f", j=P),
                    in_=batch2[:, 0 : NW - 1, :],
                )
                nc.sync.dma_start(
                    out=out_d[(NW - 1) * P : NPC, :],
                    in_=batch2[0:TAIL_ROWS, NW - 1, :],
                )

            def ag():
                nc.gpsimd.collective_compute(
                    "AllGather",
                    _ALU.bypass,
                    ins=[ag_in[:, :]],
                    outs=[table2[0:N_NODES, :]],
                    replica_groups=[list(range(N_CORES))],
                )

            if loop_R is None:
                body1()
                if include_ag:
                    ag()
                body2()
            elif loop_R < 0:
                # python-unrolled chain of -loop_R full iterations (AG incl.)
                for _ in range(-loop_R):
                    body1()
                    if include_ag:
                        ag()
                    body2()
            else:
                if include_ag:
                    body1()
                    ag()
                with tc.For_i(0, loop_R, 1):
                    body1()
                    body2()

    nc.compile()
    return nc


def _make_in_maps(x, W1, b1, W2, b2, g1, be1, g2, be2, a, pre):
    x_table, idx16, dst_rel, dinv_own, iota, plan = pre
    w2p = np.zeros((CIN, HID), np.float32)
    w2p[:HID] = W2
    vecs = np.stack([b1, g1, be1, b2, g2, be2, a]).astype(np.float32)
    in_maps = []
    for p in range(N_CORES):
        in_maps.append({
            "x_table": x_table,
            "idx16": idx16[p],
            "dst_rel": dst_rel[p],
            "dinv_own": dinv_own[p],
            "iota": iota,
            "w1": np.asarray(W1, np.float32),
            "w2p": w2p,
            "vecs": vecs,
        })
    return in_maps, plan


def kernel(x, edge_index, W1, b1, W2, b2, g1, be1, g2, be2, a):
    x = np.asarray(x, np.float32)
    edge_index = np.asarray(edge_index)
    pre = _preprocess(x, edge_index)
    in_maps, plan = _make_in_maps(
        x, np.asarray(W1), np.asarray(b1), np.asarray(W2), np.asarray(b2),
        np.asarray(g1), np.asarray(be1), np.asarray(g2), np.asarray(be2),
        np.asarray(a), pre)
    nc = build_program(plan)
    res = bass_utils.run_bass_kernel_spmd(nc, in_maps, core_ids=list(range(N_CORES)))
    out = np.concatenate([res.results[p]["out"] for p in range(N_CORES)], axis=0)
    return out.astype(np.float32)

